# revision 17
# baseline (speedup 1.0000x reference)
"""Trainium2 Bass kernel for LinearAttention (B=8, S=4096, D=512, H=8, DH=64).

Sharding: data-parallel over batch — core b processes batch element b end-to-end.

Host-side prep (inside kernel(), plain numpy marshaling):
  - x and all weights are cast to bf16 on the host (matmuls are bf16 anyway),
    halving input DMA bytes and removing every on-device cast.
  - bo' = bv @ Wo + bo is folded on the host: since out = O·Z + bv·den/(den+eps)
    and den ≈ 3e5 >> eps = 1e-6, the v-bias contributes exactly bv to the
    pre-Wo activations (to ~1e-11 relative), i.e. bv@Wo + bo after Wo.
    This removes the on-device v-bias add entirely.

Per-core pipeline (all matmul inputs bf16; PSUM f32):
  pass A (per 512-wide s-chunk, transposes one chunk ahead):
    x chunk (bf16) -> one 512KB DMA -> PE-transpose directly (1 cyc/row)
    qT = Wq^T x^T (psum), phi -> QfT [inner, s]  (bq via ACT per-partition bias)
    k  = x Wk (+bk via rank-1 ones-row matmul), phi -> Kf [s, inner]
    v  = x Wv -> copy to v' [128, pair, 128+ones]
    KV accumulated per head pair in [128, 129] psum tiles (col 128 = Ksum)
    phi staging (exp/relu) in bf16 so the DVE min/add combine runs at 2x.
  boundary: ksum_bd extraction on DVE, kvsb block extraction on ACT (parallel).
  pass B:
    den phase: denT [8,512] per slice (eps via ACT Ln bias), Z^T=exp(-ln(den+eps))
               -> ztsb [8, S] bf16, all up front right after pass A.
    main loop (per 256-wide half-slice, OW skewed one iteration behind so the
    zrep->ACT->DVE chain never stalls the PE):
      O^T per head pair (block-diag kvsb, psum f32, 2-buf)
      Zrep via selector matmul (E8) -> ACT stage -> DVE mul -> OT bf16
      out = OT^T Wo (psum, 2-buf); +bo' in the DVE psum->sbuf copy -> DMA
    output DMAs rotate across the sync/scalar/gpsimd queues.
"""

import os
import sys

import numpy as np

for _p in ("/opt/trn_rl_repo",):
    if os.path.isdir(_p) and _p not in sys.path:
        sys.path.insert(0, _p)

from contextlib import ExitStack

import ml_dtypes

import concourse.bass as bass
import concourse.mybir as mybir
import concourse.tile as tile
from concourse.bass_utils import run_bass_kernel_spmd
from concourse.masks import make_identity

B, S, D = 8, 4096, 512
H, DH = 8, 64
INNER = H * DH  # 512
EPS = 1e-6

F32 = mybir.dt.float32
BF16 = mybir.dt.bfloat16
AF = mybir.ActivationFunctionType
ALU = mybir.AluOpType

NP_BF16 = ml_dtypes.bfloat16


def _linattn_body(ctx: ExitStack, tc: "tile.TileContext", io: dict, s_total: int, reps: int = 1):
    nc = tc.nc
    NT = s_total // 128  # s-tiles
    NCH = s_total // 512  # pass-A chunks
    NSL = s_total // 512  # pass-B den slices
    NHS = s_total // 256  # pass-B half-slices

    x_d = io["x"]
    out_d = io["out"]

    singles = ctx.enter_context(tc.tile_pool(name="singles", bufs=1))

    # ---- weights: direct bf16 DMA, no casts. The critical DMA prefix is
    # strictly serialized on the sync HW queue in exact need-order
    # (x0, x1, Wk, Wv, Wq, x2, ...) — concurrent queues would fair-share the
    # ~350GB/s SDMA pool and delay x0/Wk past their first use. Wo is emitted
    # mid-pass-A on scalar; bo2 late on gpsimd (both needed much later). ----
    w_sb = {}

    def emit_weight(name, queue):
        t = singles.tile([128, 4, INNER], BF16, name=f"{name}_sb", tag=f"{name}_sb")
        queue.dma_start(out=t[:], in_=io[name].rearrange("(c p) n -> p c n", p=128))
        w_sb[name] = t

    # ---- constants on gpsimd (identity needed by the first transposes) ----
    ident = singles.tile([128, 128], BF16, name="ident", tag="ident")
    make_identity(nc, ident[:])

    # biases first on the gpsimd queue (tiny, needed by ~t=10us)
    bq_sb = singles.tile([128, 4], F32, name="bq_sb", tag="bq_sb")
    nc.gpsimd.dma_start(out=bq_sb[:], in_=io["bq"].rearrange("(c p) -> p c", p=128))
    bk_row = singles.tile([1, INNER], BF16, name="bk_row", tag="bk_row")
    nc.gpsimd.dma_start(out=bk_row[:], in_=io["bk"].rearrange("(a n) -> a n", a=1))
    bo2_rep = singles.tile([128, D], F32, name="bo2_rep", tag="bo2_rep")

    # ---- more constants (all bf16 now: direct memset/affine_select) ----
    ones_col = singles.tile([1, 128], BF16, name="ones_col", tag="ones_col")
    nc.gpsimd.memset(ones_col[:], 1.0)
    # E8 selector: zrep[p, c, :] = Z[2c + p//64, :]
    e8 = singles.tile([8, 4, 128], BF16, name="e8", tag="e8")
    nc.gpsimd.memset(e8[:], 0.0)
    nc.gpsimd.affine_select(
        out=e8[:, :, 0:64], in_=e8[:, :, 0:64], compare_op=ALU.not_equal, fill=1.0,
        base=0, pattern=[[-2, 4], [0, 64]], channel_multiplier=1,
    )
    nc.gpsimd.affine_select(
        out=e8[:, :, 64:128], in_=e8[:, :, 64:128], compare_op=ALU.not_equal, fill=1.0,
        base=-1, pattern=[[-2, 4], [0, 64]], channel_multiplier=1,
    )
    def emit_bo2():
        # bo2 replication DMA mid-pass-A on scalar: pass-B-only data, keeps
        # the 256KB partition-broadcast read off the critical DMA window
        _ap = io["bo2"]
        nc.scalar.dma_start(
            out=bo2_rep[:],
            in_=bass.AP(tensor=_ap.tensor, offset=_ap.offset, ap=[[0, 128]] + list(_ap.ap)),
        )

    # ---- persistent per-core buffers ----
    qfT = singles.tile([128, 4, s_total], BF16, name="qfT", tag="qfT")  # [inner, s]
    kvsb = singles.tile([128, 4, 128], BF16, name="kvsb", tag="kvsb")  # block-diag per pair
    nc.gpsimd.memset(kvsb[:], 0.0)
    # v' staging buffers (manual 3-deep rotation so the ones column is written once)
    vqs = [singles.tile([128, 4, 129], BF16, name=f"vq{i}", tag=f"vq{i}") for i in range(3)]
    for i in range(3):
        nc.gpsimd.memset(vqs[i][:, :, 128:129], 1.0)
    # block-diag Ksum rhs: [128, pair, 8]; pair c: rows 0-63 -> col 2c, rows 64-127 -> col 2c+1
    ksum_bd = singles.tile([128, 4, 8], BF16, name="ksum_bd", tag="ksum_bd")
    nc.gpsimd.memset(ksum_bd[:], 0.0)
    # Z^T for the whole sequence
    ztsb = singles.tile([8, s_total], BF16, name="ztsb", tag="ztsb")
    # eps as a per-partition ACT bias column
    eps_b = singles.tile([8, 1], F32, name="eps_b", tag="eps_b")
    nc.gpsimd.memset(eps_b[:], EPS)

    def _passes():
        # =================== PASS A ===================
        with ExitStack() as actx:
            x_pool = actx.enter_context(tc.tile_pool(name="x_pool", bufs=2))
            xT_pool = actx.enter_context(tc.tile_pool(name="xT_pool", bufs=2))
            er_pool = actx.enter_context(tc.tile_pool(name="er_pool", bufs=8))
            kf_pool = actx.enter_context(tc.tile_pool(name="kf_pool", bufs=3))
            ps_a = actx.enter_context(tc.tile_pool(name="ps_a", bufs=4, space="PSUM"))
            ps_acc = actx.enter_context(tc.tile_pool(name="ps_acc", bufs=1, space="PSUM"))

            # KV accumulators per head pair (one PSUM bank each — interleaved
            # accumulation groups must not share a bank):
            # kvq[j] = cols 0-127 pair j's v cols, col 128 = Ksum
            kvq = [
                ps_acc.tile([128, 129], F32, name=f"kvq_{j}", tag=f"kvq_{j}")[:]
                for j in range(4)
            ]

            xT_live = {}

            xc_live = {}

            def dma_stage(ich):
                # one 512KB bf16 DMA for the whole 512-row chunk
                xc = x_pool.tile([128, 4, D], BF16, tag="x")
                nc.sync.dma_start(
                    out=xc[:],
                    in_=x_d[ich * 512 : (ich + 1) * 512, :].rearrange(
                        "(t p) d -> p t d", p=128
                    ),
                )
                xc_live[ich] = xc

            def transpose_work(ich):
                xc = xc_live.pop(ich)
                xT_t = xT_pool.tile([128, 4, 512], BF16, tag="xT")
                # ---- transpose directly from the DMA'd bf16 (1 cyc/row) ----
                for it in range(4):
                    xps = ps_a.tile([128, 4, 128], BF16, tag="ps")
                    for c in range(4):
                        nc.tensor.transpose(
                            xps[:, c, :], xc[:, it, c * 128 : (c + 1) * 128], ident[:]
                        )
                    nc.vector.tensor_copy(out=xT_t[:, :, it * 128 : (it + 1) * 128], in_=xps[:])
                xT_live[ich] = xT_t

            def q_stage(ich, xT_t):
                # ---- qT = Wq^T xT, phi -> QfT ----
                # Emitted AFTER the K/V/KV pipeline: Q's phi chain is consumed
                # only in pass B (latency-tolerant), so keeping its ACT ops out
                # of the queue ahead of K's latency-critical exp/relu tightens
                # the kf -> KV chain.
                for ci in range(4):
                    qps = ps_a.tile([128, 512], F32, tag="ps")
                    for cd in range(4):
                        nc.tensor.matmul(
                            qps[:],
                            lhsT=w_sb["Wq"][:, cd, ci * 128 : (ci + 1) * 128],
                            rhs=xT_t[:, cd, :],
                            start=(cd == 0),
                            stop=(cd == 3),
                        )
                    e_t = er_pool.tile([128, 512], BF16, tag="er")
                    r_t = er_pool.tile([128, 512], BF16, tag="er")
                    nc.scalar.activation(e_t[:], qps[:], AF.Exp, bias=bq_sb[:, ci : ci + 1], scale=1.0)
                    nc.scalar.activation(r_t[:], qps[:], AF.Relu, bias=bq_sb[:, ci : ci + 1], scale=1.0)
                    # phi = min(exp(x),1) + relu(x)
                    nc.vector.scalar_tensor_tensor(
                        out=qfT[:, ci, ich * 512 : (ich + 1) * 512],
                        in0=e_t[:],
                        scalar=1.0,
                        in1=r_t[:],
                        op0=ALU.min,
                        op1=ALU.add,
                    )

            def extraction():
                # Ksum on DVE (den needs it first), KV blocks on ACT (parallel
                # engines). Emitted before the last q_stage so pass B isn't
                # queued behind its ACT/DVE work.
                for c in range(4):
                    for half in range(2):
                        nc.vector.tensor_copy(
                            out=ksum_bd[
                                half * 64 : (half + 1) * 64, c, 2 * c + half : 2 * c + half + 1
                            ],
                            in_=kvq[c][half * 64 : (half + 1) * 64, 128:129],
                        )
                for h in range(H):
                    j, rh = h // 2, (h % 2) * 64
                    nc.scalar.activation(
                        kvsb[rh : rh + 64, j, rh : rh + 64],
                        kvq[j][rh : rh + 64, rh : rh + 64],
                        AF.Copy,
                    )

            def qkv_stage(ich):
                last = ich == NCH - 1
                xT_t = xT_live.pop(ich)
                # ---- k, v, KV accumulation per s-tile, one-tile K lookahead:
                # K(it+1)'s matmuls fill the PE while kf(it) transits the
                # ACT exp/relu -> DVE stt chain, so the KV matmuls never stall
                # (stalls also drop the PE out of its boosted p-state).
                er_live, kf_live = {}, {}

                def emit_K_mm(it):
                    # k (natural layout) + bias via ones-row matmul
                    kps = ps_a.tile([128, 512], F32, tag="ps")
                    for cd in range(4):
                        nc.tensor.matmul(
                            kps[:],
                            lhsT=xT_t[:, cd, it * 128 : (it + 1) * 128],
                            rhs=w_sb["Wk"][:, cd, :],
                            start=(cd == 0),
                            stop=False,
                        )
                    nc.tensor.matmul(
                        kps[:], lhsT=ones_col[:], rhs=bk_row[:], start=False, stop=True
                    )
                    e_t = er_pool.tile([128, 512], BF16, tag="er")
                    r_t = er_pool.tile([128, 512], BF16, tag="er")
                    nc.scalar.activation(e_t[:], kps[:], AF.Exp)
                    nc.scalar.activation(r_t[:], kps[:], AF.Relu)
                    er_live[it] = (e_t, r_t)

                def emit_K_stt(it):
                    e_t, r_t = er_live.pop(it)
                    kf = kf_pool.tile([128, 512], BF16, tag="kf")
                    nc.vector.scalar_tensor_tensor(
                        out=kf[:], in0=e_t[:], scalar=1.0, in1=r_t[:], op0=ALU.min, op1=ALU.add
                    )
                    kf_live[it] = kf

                def emit_VKV(it):
                    ist = ich * 4 + it
                    first, last = (ist == 0), (ist == NT - 1)
                    # v (natural); psum->sbuf copy on DVE (no bias — folded on host)
                    vps = ps_a.tile([128, 512], F32, tag="ps")
                    for cd in range(4):
                        nc.tensor.matmul(
                            vps[:],
                            lhsT=xT_t[:, cd, it * 128 : (it + 1) * 128],
                            rhs=w_sb["Wv"][:, cd, :],
                            start=(cd == 0),
                            stop=(cd == 3),
                        )
                    vq = vqs[ist % 3]
                    nc.vector.tensor_copy(
                        out=vq[:, :, 0:128],
                        in_=vps[:].rearrange("p (g n) -> p g n", g=4),
                    )
                    kf = kf_live.pop(it)
                    # KV accumulation per head pair
                    for j in range(4):
                        nc.tensor.matmul(
                            kvq[j],
                            lhsT=kf[:, j * 128 : (j + 1) * 128],
                            rhs=vq[:, j, :],
                            start=first,
                            stop=last,
                        )

                # For the LAST chunk, q runs FIRST: its ACT exp/relu drain the
                # ps_a psum banks early, so pass B's psum pools (which inherit
                # those banks) aren't WAR-blocked behind a late q ACT chain.
                if last:
                    q_stage(ich, xT_t)
                emit_K_mm(0)
                emit_K_stt(0)
                for it in range(4):
                    if it + 1 < 4:
                        emit_K_mm(it + 1)
                    emit_VKV(it)
                    if it == 0 and ich >= 1 and ich + 1 < NCH:
                        # next chunk's transposes mid-chunk: their psum-bank
                        # WAR waits overlap the k/v matmul stream
                        transpose_work(ich + 1)
                    if it + 1 < 4:
                        emit_K_stt(it + 1)
                if last:
                    extraction()
                else:
                    q_stage(ich, xT_t)

            # software pipeline: x DMAs and weight DMAs slot into the sync
            # queue in exact PE need-order: x0, x1, Wk, Wv, Wq, x2, ...
            for ich in range(NCH + 1):
                if ich < NCH:
                    dma_stage(ich)
                if ich == 0:
                    transpose_work(0)
                if ich == 1:
                    transpose_work(1)
                    emit_weight("Wk", nc.sync)
                    emit_weight("Wv", nc.sync)
                    emit_weight("Wq", nc.sync)
                if ich == 2:
                    # late-emitted Wo + bo2: their transfers run mid-pass-A on
                    # the otherwise DMA-free scalar queue
                    emit_weight("Wo", nc.scalar)
                    emit_bo2()
                if ich >= 1:
                    qkv_stage(ich - 1)

        # ======================= PASS B =======================
        # den slices are interleaved into the main loop (two slices ahead of
        # their half-slice consumers) so the ACT Ln/Exp chain and the PE den
        # matmuls spread across the loop instead of forming a serial prologue.
        with ExitStack() as bctx:
            dz_ps = bctx.enter_context(tc.tile_pool(name="dz_ps", bufs=2, space="PSUM"))
            ot_ps = bctx.enter_context(tc.tile_pool(name="ot_ps", bufs=2, space="PSUM"))
            zr_ps = bctx.enter_context(tc.tile_pool(name="zr_ps", bufs=2, space="PSUM"))
            ow_ps = bctx.enter_context(tc.tile_pool(name="ow_ps", bufs=2, space="PSUM"))
            lnt_pool = bctx.enter_context(tc.tile_pool(name="lnt_pool", bufs=2))
            zsb_pool = bctx.enter_context(tc.tile_pool(name="zsb_pool", bufs=2))
            osb_pool = bctx.enter_context(tc.tile_pool(name="osb_pool", bufs=4))
            out_pool = bctx.enter_context(tc.tile_pool(name="out_pool", bufs=4))

            otsb_live = {}
            out_queues = [nc.sync, nc.scalar]

            def den_stage(isl):
                # denT [8, 512] = sum_c ksum_bd[:,c,:]^T @ qfT[:,c,slice]
                dz = dz_ps.tile([8, 512], F32, tag="dz")
                for c in range(4):
                    nc.tensor.matmul(
                        dz[:],
                        lhsT=ksum_bd[:, c, :],
                        rhs=qfT[:, c, isl * 512 : (isl + 1) * 512],
                        start=(c == 0),
                        stop=(c == 3),
                    )
                # Z = 1/(den+eps) as exp(-ln(den+eps)) on ACT; eps folds into
                # the Ln bias (Reciprocal is banned on ACT; DVE recip too slow)
                lntmp = lnt_pool.tile([8, 512], F32, tag="lntmp")
                nc.scalar.activation(lntmp[:], dz[:], AF.Ln, bias=eps_b[:])
                nc.scalar.activation(
                    ztsb[:, isl * 512 : (isl + 1) * 512], lntmp[:], AF.Exp, scale=-1.0
                )

            def stage1(iq):
                s0 = iq * 128
                # O^T per head pair (block-diagonal KV)
                otps = ot_ps.tile([128, 4, 128], F32, tag="ot")
                for c in range(4):
                    nc.tensor.matmul(
                        otps[:, c, :],
                        lhsT=kvsb[:, c, :],
                        rhs=qfT[:, c, s0 : s0 + 128],
                        start=True,
                        stop=True,
                    )
                # Zrep: replicate Z rows across head d-partitions (E8 selector)
                zrep = zr_ps.tile([128, 4, 128], F32, tag="zrep")
                for c in range(4):
                    nc.tensor.matmul(
                        zrep[:, c, :],
                        lhsT=e8[:, c, :],
                        rhs=ztsb[:, s0 : s0 + 128],
                        start=True,
                        stop=True,
                    )
                # DVE may read only one PSUM operand per op: stage zrep via ACT
                zrep_sb = zsb_pool.tile([128, 4, 128], BF16, tag="zrep_sb")
                nc.scalar.activation(zrep_sb[:], zrep[:], AF.Copy)
                otsb = osb_pool.tile([128, 4, 128], BF16, tag="otsb")
                nc.vector.tensor_mul(out=otsb[:], in0=otps[:], in1=zrep_sb[:])
                otsb_live[iq] = otsb

            def stage2(iq):
                otsb = otsb_live.pop(iq)
                # out = OT^T Wo; +bo' fused into the psum->sbuf copy
                owps = ow_ps.tile([128, 512], F32, tag="ow")
                for c in range(4):
                    nc.tensor.matmul(
                        owps[:],
                        lhsT=otsb[:, c, :],
                        rhs=w_sb["Wo"][:, c, :],
                        start=(c == 0),
                        stop=(c == 3),
                    )
                outt = out_pool.tile([128, 512], F32, tag="outt")
                nc.vector.tensor_add(out=outt[:], in0=owps[:], in1=bo2_rep[:])
                outq = out_queues[iq % 2]
                outq.dma_start(out=out_d[iq * 128 : (iq + 1) * 128, :], in_=outt[:])

            # quarter-slice main loop (128-wide: 1 PSUM bank per tile so
            # everything double-buffers within the 8 banks); OW skewed two
            # iterations behind OT/zrep so the zrep->ACT->DVE chain overlaps
            NQS = s_total // 128
            den_stage(0)
            den_stage(1)
            for iq in range(NQS + 2):
                if iq < NQS:
                    stage1(iq)
                if iq % 4 == 3 and (iq // 4 + 2) < NSL:
                    den_stage(iq // 4 + 2)
                if iq >= 2:
                    stage2(iq - 2)

    if reps == 1:
        _passes()
    else:
        with tc.For_i(0, reps, 1):
            _passes()


def _legalize_waits(nc: "bass.Bass", max_waits: int = 1) -> int:
    """This toolchain's walrus allows at most ONE sync wait per instruction.

    Tile's scheduler attaches several; hoist the extras into standalone
    event-semaphore (pure wait) instructions on the same engine, placed
    immediately before the original — identical blocking semantics since
    waits execute in stream order on the issuing sequencer.
    """
    n_split = 0
    for func in nc.m.functions:
        for block in func.blocks:
            new_insts = []
            for inst in block.instructions:
                si = getattr(inst, "sync_info", None)
                waits = list(si.on_wait) if (si and si.on_wait) else []
                if len(waits) > max_waits:
                    extra, keep = waits[:-max_waits], waits[-max_waits:]
                    for j, w in enumerate(extra):
                        ev = mybir.InstEventSemaphore(
                            name=f"{inst.name}_lw{j}",
                            engine=inst.engine,
                            ins=[],
                            outs=[],
                            sync_info=mybir.SyncInfo(on_wait=[w], on_update=[]),
                        )
                        new_insts.append(ev)
                        n_split += 1
                    si.on_wait = keep
                new_insts.append(inst)
            block.instructions[:] = new_insts
    return n_split


def build_program(s_total: int = S, reps: int = 1) -> "bass.Bass":
    nc = bass.Bass("TRN2", target_bir_lowering=False, debug=False, num_devices=B)
    io = {
        "x": nc.dram_tensor("x", [s_total, D], BF16, kind="ExternalInput").ap(),
        "Wq": nc.dram_tensor("Wq", [D, INNER], BF16, kind="ExternalInput").ap(),
        "bq": nc.dram_tensor("bq", [INNER], F32, kind="ExternalInput").ap(),
        "Wk": nc.dram_tensor("Wk", [D, INNER], BF16, kind="ExternalInput").ap(),
        "bk": nc.dram_tensor("bk", [INNER], BF16, kind="ExternalInput").ap(),
        "Wv": nc.dram_tensor("Wv", [D, INNER], BF16, kind="ExternalInput").ap(),
        "Wo": nc.dram_tensor("Wo", [INNER, D], BF16, kind="ExternalInput").ap(),
        "bo2": nc.dram_tensor("bo2", [D], F32, kind="ExternalInput").ap(),
        "out": nc.dram_tensor("out", [s_total, D], F32, kind="ExternalOutput").ap(),
    }
    with tile.TileContext(nc) as tc:
        with ExitStack() as ctx:
            _linattn_body(ctx, tc, io, s_total, reps=reps)
    return nc


_PROGRAM_CACHE: dict = {}


def _get_program(s_total: int = S) -> "bass.Bass":
    if s_total not in _PROGRAM_CACHE:
        nc = build_program(s_total)
        _legalize_waits(nc)
        _PROGRAM_CACHE[s_total] = nc
    return _PROGRAM_CACHE[s_total]


def _in_maps(inputs: dict) -> list:
    x_bf = np.asarray(inputs["x"]).astype(NP_BF16)
    w_bf = {
        nm: np.ascontiguousarray(np.asarray(inputs[nm]).astype(NP_BF16))
        for nm in ("Wq", "Wk", "Wv", "Wo")
    }
    bq = np.ascontiguousarray(inputs["bq"], dtype=np.float32)
    bk = np.ascontiguousarray(np.asarray(inputs["bk"]).astype(NP_BF16))
    bo2 = (
        np.asarray(inputs["bv"], dtype=np.float64) @ np.asarray(inputs["Wo"], dtype=np.float64)
        + np.asarray(inputs["bo"], dtype=np.float64)
    ).astype(np.float32)
    maps = []
    for b in range(B):
        m = {"x": np.ascontiguousarray(x_bf[b]), "bq": bq, "bk": bk, "bo2": bo2}
        m.update(w_bf)
        maps.append(m)
    return maps


def run_hw(inputs: dict, trace: bool = False, **kwargs):
    """Run on the 8 NeuronCores. Returns (out [B,S,D], BassKernelResults)."""
    nc = _get_program(S)
    res = run_bass_kernel_spmd(nc, _in_maps(inputs), list(range(B)), trace=trace, **kwargs)
    out = np.stack([res.results[b]["out"] for b in range(B)], axis=0)
    return out, res


def kernel(**inputs) -> np.ndarray:
    out, _ = run_hw(inputs, trace=False)
    return out


def bench_hw(inputs: dict, iters: int = 20, nc_override=None):
    """Time repeated NEFF executions with device-resident inputs.

    Returns (per_iter_ns, out[B,S,D] from the first run). Uses the same
    shard_map lowering as run_bass_via_pjrt, without donation so input
    buffers can be reused across timed calls.
    """
    import time as _time

    import jax
    from jax.sharding import Mesh, NamedSharding, PartitionSpec
    from jax.experimental.shard_map import shard_map

    from concourse import bass2jax
    from concourse.bass2jax import _bass_exec_p, install_neuronx_cc_hook

    install_neuronx_cc_hook()
    nc = nc_override if nc_override is not None else _get_program(S)
    in_maps = _in_maps(inputs)

    partition_name = nc.partition_id_tensor.name if nc.partition_id_tensor else None
    in_names, out_names, out_avals = [], [], []
    for alloc in nc.m.functions[0].allocations:
        if not isinstance(alloc, mybir.MemoryLocationSet):
            continue
        name = alloc.memorylocations[0].name
        if alloc.kind == "ExternalInput":
            if name != partition_name:
                in_names.append(name)
        elif alloc.kind == "ExternalOutput":
            out_names.append(name)
            out_avals.append(
                jax.core.ShapedArray(tuple(alloc.tensor_shape), mybir.dt.np(alloc.dtype))
            )
    n_params = len(in_names)
    all_in_names = in_names + out_names
    if partition_name is not None:
        all_in_names = all_in_names + [partition_name]

    def _body(*args):
        operands = list(args)
        if partition_name is not None:
            operands.append(bass2jax.partition_id_tensor())
        outs = _bass_exec_p.bind(
            *operands,
            out_avals=tuple(out_avals),
            in_names=tuple(all_in_names),
            out_names=tuple(out_names),
            lowering_input_output_aliases=(),
            sim_require_finite=True,
            sim_require_nnan=True,
            nc=nc,
        )
        return tuple(outs)

    devices = jax.devices()[:B]
    mesh = Mesh(np.asarray(devices), ("core",))
    n_outs = len(out_names)
    in_specs = (PartitionSpec("core"),) * (n_params + n_outs)
    out_specs = (PartitionSpec("core"),) * n_outs
    fn = jax.jit(
        shard_map(_body, mesh=mesh, in_specs=in_specs, out_specs=out_specs, check_rep=False)
    )

    sh = NamedSharding(mesh, PartitionSpec("core"))
    concat_in = [
        jax.device_put(
            np.concatenate([np.asarray(in_maps[c][nm])[None] for c in range(B)], axis=0).reshape(
                B * np.asarray(in_maps[0][nm]).shape[0], *np.asarray(in_maps[0][nm]).shape[1:]
            ),
            sh,
        )
        for nm in in_names
    ]
    concat_zeros = [
        jax.device_put(np.zeros((B * a.shape[0], *a.shape[1:]), a.dtype), sh) for a in out_avals
    ]

    out = fn(*concat_in, *concat_zeros)
    jax.block_until_ready(out)
    first = np.asarray(out[0]).reshape(B, *out_avals[0].shape)

    def timed(f, n):
        t0 = _time.perf_counter()
        for _ in range(n):
            r = f(*concat_in, *concat_zeros)
        jax.block_until_ready(r)
        return (_time.perf_counter() - t0) / n

    timed(fn, 3)
    t = min(timed(fn, max(5, iters // 2)) for _ in range(4))
    return int(t * 1e9), first


def build_copy_program(s_total: int = S) -> "bass.Bass":
    """Same I/O signature as the real program, near-zero work: out = x upcast."""
    nc = bass.Bass("TRN2", target_bir_lowering=False, debug=False, num_devices=B)
    io = {}
    io["x"] = nc.dram_tensor("x", [s_total, D], BF16, kind="ExternalInput").ap()
    for nm, shp, dt in (
        ("Wq", [D, INNER], BF16), ("bq", [INNER], F32), ("Wk", [D, INNER], BF16),
        ("bk", [INNER], BF16), ("Wv", [D, INNER], BF16), ("Wo", [INNER, D], BF16),
        ("bo2", [D], F32),
    ):
        io[nm] = nc.dram_tensor(nm, shp, dt, kind="ExternalInput").ap()
    out_d = nc.dram_tensor("out", [s_total, D], F32, kind="ExternalOutput").ap()
    from contextlib import ExitStack as _ES
    with tile.TileContext(nc) as tc:
        with _ES() as ctx:
            pool = ctx.enter_context(tc.tile_pool(name="cp", bufs=4))
            for i in range(s_total // 128):
                t = pool.tile([128, D], BF16, tag="cp")
                t2 = pool.tile([128, D], F32, tag="cp2")
                sl = slice(i * 128, (i + 1) * 128)
                nc.sync.dma_start(out=t[:], in_=io["x"][sl])
                nc.vector.tensor_copy(out=t2[:], in_=t[:])
                nc.sync.dma_start(out=out_d[sl], in_=t2[:])
    _legalize_waits(nc)
    return nc


# revision 28
# speedup vs baseline: 40.3727x; 40.3727x over previous
"""Trainium2 Bass kernel for LinearAttention (B=8, S=4096, D=512, H=8, DH=64).

Sharding: data-parallel over batch — core b processes batch element b end-to-end.

Host-side prep (inside kernel(), plain numpy marshaling):
  - x and all weights are cast to bf16 on the host (matmuls are bf16 anyway),
    halving input DMA bytes and removing every on-device cast.
  - bo' = bv @ Wo + bo is folded on the host: since out = O·Z + bv·den/(den+eps)
    and den ≈ 3e5 >> eps = 1e-6, the v-bias contributes exactly bv to the
    pre-Wo activations (to ~1e-11 relative), i.e. bv@Wo + bo after Wo.
    This removes the on-device v-bias add entirely.

Per-core pipeline (all matmul inputs bf16; PSUM f32):
  pass A (per 512-wide s-chunk, transposes one chunk ahead):
    x chunk (bf16) -> one 512KB DMA -> PE-transpose directly (1 cyc/row)
    qT = Wq^T x^T (psum), phi -> QfT [inner, s]  (bq via ACT per-partition bias)
    k  = x Wk (+bk via rank-1 ones-row matmul), phi -> Kf [s, inner]
    v  = x Wv -> copy to v' [128, pair, 128+ones]
    KV accumulated per head pair in [128, 129] psum tiles (col 128 = Ksum)
    phi staging (exp/relu) in bf16 so the DVE min/add combine runs at 2x.
  boundary: ksum_bd extraction on DVE, kvsb block extraction on ACT (parallel).
  pass B:
    den phase: denT [8,512] per slice (eps via ACT Ln bias), Z^T=exp(-ln(den+eps))
               -> ztsb [8, S] bf16, all up front right after pass A.
    main loop (per 256-wide half-slice, OW skewed one iteration behind so the
    zrep->ACT->DVE chain never stalls the PE):
      O^T per head pair (block-diag kvsb, psum f32, 2-buf)
      Zrep via selector matmul (E8) -> ACT stage -> DVE mul -> OT bf16
      out = OT^T Wo (psum, 2-buf); +bo' in the DVE psum->sbuf copy -> DMA
    output DMAs rotate across the sync/scalar/gpsimd queues.
"""

import os
import sys

import numpy as np

for _p in ("/opt/trn_rl_repo",):
    if os.path.isdir(_p) and _p not in sys.path:
        sys.path.insert(0, _p)

from contextlib import ExitStack

import ml_dtypes

import concourse.bass as bass
import concourse.mybir as mybir
import concourse.tile as tile
from concourse.bass_utils import run_bass_kernel_spmd
from concourse.masks import make_identity

B, S, D = 8, 4096, 512
H, DH = 8, 64
INNER = H * DH  # 512
EPS = 1e-6

F32 = mybir.dt.float32
BF16 = mybir.dt.bfloat16
AF = mybir.ActivationFunctionType
ALU = mybir.AluOpType

NP_BF16 = ml_dtypes.bfloat16


def _linattn_body(ctx: ExitStack, tc: "tile.TileContext", io: dict, s_total: int, reps: int = 1):
    nc = tc.nc
    NT = s_total // 128  # s-tiles
    NCH = s_total // 512  # pass-A chunks
    NSL = s_total // 512  # pass-B den slices
    NHS = s_total // 256  # pass-B half-slices

    x_d = io["xT"]
    out_d = io["out"]

    singles = ctx.enter_context(tc.tile_pool(name="singles", bufs=1))

    # ---- weights: direct bf16 DMA, no casts. The critical DMA prefix is
    # strictly serialized on the sync HW queue in exact need-order
    # (x0, x1, Wk, Wv, Wq, x2, ...) — concurrent queues would fair-share the
    # ~350GB/s SDMA pool and delay x0/Wk past their first use. Wo is emitted
    # mid-pass-A on scalar; bo2 late on gpsimd (both needed much later). ----
    w_sb = {}

    def emit_weight(name, queue):
        t = singles.tile([128, 4, INNER], BF16, name=f"{name}_sb", tag=f"{name}_sb")
        queue.dma_start(out=t[:], in_=io[name].rearrange("(c p) n -> p c n", p=128))
        w_sb[name] = t

    # ---- identity: only used by the PE warm-up transposes ----
    ident = singles.tile([128, 128], BF16, name="ident", tag="ident")
    make_identity(nc, ident[:])

    # biases first on the gpsimd queue (tiny, needed by ~t=10us)
    bq_sb = singles.tile([128, 4], F32, name="bq_sb", tag="bq_sb")
    nc.gpsimd.dma_start(out=bq_sb[:], in_=io["bq"].rearrange("(c p) -> p c", p=128))
    bk_row = singles.tile([1, INNER], BF16, name="bk_row", tag="bk_row")
    nc.gpsimd.dma_start(out=bk_row[:], in_=io["bk"].rearrange("(a n) -> a n", a=1))
    bo2_rep = singles.tile([128, D], F32, name="bo2_rep", tag="bo2_rep")

    # ---- more constants (all bf16 now: direct memset/affine_select) ----
    ones_col = singles.tile([1, 128], BF16, name="ones_col", tag="ones_col")
    nc.gpsimd.memset(ones_col[:], 1.0)
    # E8 selector: zrep[p, c, :] = Z[2c + p//64, :]
    e8 = singles.tile([8, 4, 128], BF16, name="e8", tag="e8")
    nc.gpsimd.memset(e8[:], 0.0)
    nc.gpsimd.affine_select(
        out=e8[:, :, 0:64], in_=e8[:, :, 0:64], compare_op=ALU.not_equal, fill=1.0,
        base=0, pattern=[[-2, 4], [0, 64]], channel_multiplier=1,
    )
    nc.gpsimd.affine_select(
        out=e8[:, :, 64:128], in_=e8[:, :, 64:128], compare_op=ALU.not_equal, fill=1.0,
        base=-1, pattern=[[-2, 4], [0, 64]], channel_multiplier=1,
    )
    def emit_bo2():
        # bo2 replication DMA mid-pass-A on scalar: pass-B-only data, keeps
        # the 256KB partition-broadcast read off the critical DMA window
        _ap = io["bo2"]
        nc.scalar.dma_start(
            out=bo2_rep[:],
            in_=bass.AP(tensor=_ap.tensor, offset=_ap.offset, ap=[[0, 128]] + list(_ap.ap)),
        )

    # ---- persistent per-core buffers ----
    qfT = singles.tile([128, 4, s_total], BF16, name="qfT", tag="qfT")  # [inner, s]
    kvsb = singles.tile([128, 4, 128], BF16, name="kvsb", tag="kvsb")  # block-diag per pair
    nc.gpsimd.memset(kvsb[:], 0.0)
    # v' staging buffers (manual 3-deep rotation so the ones column is written once)
    vqs = [singles.tile([128, 4, 129], BF16, name=f"vq{i}", tag=f"vq{i}") for i in range(3)]
    for i in range(3):
        nc.gpsimd.memset(vqs[i][:, :, 128:129], 1.0)
    # block-diag Ksum rhs: [128, pair, 8]; pair c: rows 0-63 -> col 2c, rows 64-127 -> col 2c+1
    ksum_bd = singles.tile([128, 4, 8], BF16, name="ksum_bd", tag="ksum_bd")
    nc.gpsimd.memset(ksum_bd[:], 0.0)
    # Z^T for the whole sequence
    ztsb = singles.tile([8, s_total], BF16, name="ztsb", tag="ztsb")
    # eps as a per-partition ACT bias column
    eps_b = singles.tile([8, 1], F32, name="eps_b", tag="eps_b")
    nc.gpsimd.memset(eps_b[:], EPS)

    def _passes():
        # =================== PASS A ===================
        with ExitStack() as actx:
            xT_pool = actx.enter_context(tc.tile_pool(name="xT_pool", bufs=3))
            er_pool = actx.enter_context(tc.tile_pool(name="er_pool", bufs=8))
            kf_pool = actx.enter_context(tc.tile_pool(name="kf_pool", bufs=3))
            ps_a = actx.enter_context(tc.tile_pool(name="ps_a", bufs=4, space="PSUM"))
            ps_acc = actx.enter_context(tc.tile_pool(name="ps_acc", bufs=1, space="PSUM"))

            # KV accumulators per head pair (one PSUM bank each — interleaved
            # accumulation groups must not share a bank):
            # kvq[j] = cols 0-127 pair j's v cols, col 128 = Ksum
            kvq = [
                ps_acc.tile([128, 129], F32, name=f"kvq_{j}", tag=f"kvq_{j}")[:]
                for j in range(4)
            ]

            xT_live = {}

            def dma_stage(ich):
                # x arrives pre-transposed from the host: one 512KB bf16 DMA
                # lands the chunk directly in matmul-ready [d, s] layout
                xT_t = xT_pool.tile([128, 4, 512], BF16, tag="xT")
                nc.sync.dma_start(
                    out=xT_t[:],
                    in_=x_d[:, ich * 512 : (ich + 1) * 512].rearrange(
                        "(c p) s -> p c s", p=128
                    ),
                )
                xT_live[ich] = xT_t

            def warmup_mms(n=48):
                # junk identity matmuls keep the PE HAM activity window busy
                # from ~7us so the first real matmuls run at 2.4GHz (reuses
                # the ps tag so the pool doesn't grow)
                for _ in range(n):
                    wps = ps_a.tile([128, 512], F32, tag="ps")
                    nc.tensor.matmul(
                        wps[:, 0:128], lhsT=ident[:], rhs=ident[:], start=True, stop=True
                    )

            def q_stage(ich, xT_t):
                # ---- qT = Wq^T xT, phi -> QfT ----
                # Emitted AFTER the K/V/KV pipeline: Q's phi chain is consumed
                # only in pass B (latency-tolerant), so keeping its ACT ops out
                # of the queue ahead of K's latency-critical exp/relu tightens
                # the kf -> KV chain.
                for ci in range(4):
                    qps = ps_a.tile([128, 512], F32, tag="ps")
                    for cd in range(4):
                        nc.tensor.matmul(
                            qps[:],
                            lhsT=w_sb["Wq"][:, cd, ci * 128 : (ci + 1) * 128],
                            rhs=xT_t[:, cd, :],
                            start=(cd == 0),
                            stop=(cd == 3),
                        )
                    e_t = er_pool.tile([128, 512], BF16, tag="er")
                    r_t = er_pool.tile([128, 512], BF16, tag="er")
                    nc.scalar.activation(e_t[:], qps[:], AF.Exp, bias=bq_sb[:, ci : ci + 1], scale=1.0)
                    nc.scalar.activation(r_t[:], qps[:], AF.Relu, bias=bq_sb[:, ci : ci + 1], scale=1.0)
                    # phi = min(exp(x),1) + relu(x)
                    nc.vector.scalar_tensor_tensor(
                        out=qfT[:, ci, ich * 512 : (ich + 1) * 512],
                        in0=e_t[:],
                        scalar=1.0,
                        in1=r_t[:],
                        op0=ALU.min,
                        op1=ALU.add,
                    )

            def extraction():
                # Ksum on DVE (den needs it first), KV blocks on ACT (parallel
                # engines). Emitted before the last q_stage so pass B isn't
                # queued behind its ACT/DVE work.
                for c in range(4):
                    for half in range(2):
                        nc.vector.tensor_copy(
                            out=ksum_bd[
                                half * 64 : (half + 1) * 64, c, 2 * c + half : 2 * c + half + 1
                            ],
                            in_=kvq[c][half * 64 : (half + 1) * 64, 128:129],
                        )
                for h in range(H):
                    j, rh = h // 2, (h % 2) * 64
                    nc.scalar.activation(
                        kvsb[rh : rh + 64, j, rh : rh + 64],
                        kvq[j][rh : rh + 64, rh : rh + 64],
                        AF.Copy,
                    )

            def qkv_stage(ich):
                last = ich == NCH - 1
                xT_t = xT_live.pop(ich)
                # ---- k, v, KV accumulation per s-tile, one-tile K lookahead:
                # K(it+1)'s matmuls fill the PE while kf(it) transits the
                # ACT exp/relu -> DVE stt chain, so the KV matmuls never stall
                # (stalls also drop the PE out of its boosted p-state).
                er_live, kf_live = {}, {}

                def emit_K_mm(it):
                    # k (natural layout) + bias via ones-row matmul
                    kps = ps_a.tile([128, 512], F32, tag="ps")
                    for cd in range(4):
                        nc.tensor.matmul(
                            kps[:],
                            lhsT=xT_t[:, cd, it * 128 : (it + 1) * 128],
                            rhs=w_sb["Wk"][:, cd, :],
                            start=(cd == 0),
                            stop=False,
                        )
                    nc.tensor.matmul(
                        kps[:], lhsT=ones_col[:], rhs=bk_row[:], start=False, stop=True
                    )
                    e_t = er_pool.tile([128, 512], BF16, tag="er")
                    r_t = er_pool.tile([128, 512], BF16, tag="er")
                    nc.scalar.activation(e_t[:], kps[:], AF.Exp)
                    nc.scalar.activation(r_t[:], kps[:], AF.Relu)
                    er_live[it] = (e_t, r_t)

                def emit_K_stt(it):
                    e_t, r_t = er_live.pop(it)
                    kf = kf_pool.tile([128, 512], BF16, tag="kf")
                    nc.vector.scalar_tensor_tensor(
                        out=kf[:], in0=e_t[:], scalar=1.0, in1=r_t[:], op0=ALU.min, op1=ALU.add
                    )
                    kf_live[it] = kf

                def emit_VKV(it):
                    ist = ich * 4 + it
                    first, last = (ist == 0), (ist == NT - 1)
                    # v (natural); psum->sbuf copy on DVE (no bias — folded on host)
                    vps = ps_a.tile([128, 512], F32, tag="ps")
                    for cd in range(4):
                        nc.tensor.matmul(
                            vps[:],
                            lhsT=xT_t[:, cd, it * 128 : (it + 1) * 128],
                            rhs=w_sb["Wv"][:, cd, :],
                            start=(cd == 0),
                            stop=(cd == 3),
                        )
                    vq = vqs[ist % 3]
                    nc.vector.tensor_copy(
                        out=vq[:, :, 0:128],
                        in_=vps[:].rearrange("p (g n) -> p g n", g=4),
                    )
                    kf = kf_live.pop(it)
                    # KV accumulation per head pair
                    for j in range(4):
                        nc.tensor.matmul(
                            kvq[j],
                            lhsT=kf[:, j * 128 : (j + 1) * 128],
                            rhs=vq[:, j, :],
                            start=first,
                            stop=last,
                        )

                # For the LAST chunk, q runs FIRST: its ACT exp/relu drain the
                # ps_a psum banks early, so pass B's psum pools (which inherit
                # those banks) aren't WAR-blocked behind a late q ACT chain.
                if last:
                    q_stage(ich, xT_t)
                emit_K_mm(0)
                emit_K_stt(0)
                for it in range(4):
                    if it + 1 < 4:
                        emit_K_mm(it + 1)
                    emit_VKV(it)
                    if it + 1 < 4:
                        emit_K_stt(it + 1)
                if last:
                    extraction()
                else:
                    q_stage(ich, xT_t)

            # software pipeline: x DMAs and weight DMAs slot into the sync
            # queue in exact PE need-order: xT0, Wk, Wv, Wq, xT1, xT2, ...
            for ich in range(NCH + 1):
                if ich < NCH:
                    dma_stage(ich)
                if ich == 0:
                    emit_weight("Wk", nc.sync)
                    emit_weight("Wv", nc.sync)
                    emit_weight("Wq", nc.sync)
                    warmup_mms()
                if ich == 2:
                    # late-emitted Wo + bo2: their transfers run mid-pass-A on
                    # the otherwise DMA-free scalar queue
                    emit_weight("Wo", nc.scalar)
                    emit_bo2()
                if ich >= 1:
                    qkv_stage(ich - 1)

        # ======================= PASS B =======================
        # den slices are interleaved into the main loop (two slices ahead of
        # their half-slice consumers) so the ACT Ln/Exp chain and the PE den
        # matmuls spread across the loop instead of forming a serial prologue.
        with ExitStack() as bctx:
            dz_ps = bctx.enter_context(tc.tile_pool(name="dz_ps", bufs=2, space="PSUM"))
            ot_ps = bctx.enter_context(tc.tile_pool(name="ot_ps", bufs=2, space="PSUM"))
            zr_ps = bctx.enter_context(tc.tile_pool(name="zr_ps", bufs=2, space="PSUM"))
            ow_ps = bctx.enter_context(tc.tile_pool(name="ow_ps", bufs=2, space="PSUM"))
            lnt_pool = bctx.enter_context(tc.tile_pool(name="lnt_pool", bufs=2))
            zsb_pool = bctx.enter_context(tc.tile_pool(name="zsb_pool", bufs=2))
            osb_pool = bctx.enter_context(tc.tile_pool(name="osb_pool", bufs=4))
            out_pool = bctx.enter_context(tc.tile_pool(name="out_pool", bufs=4))

            otsb_live = {}
            out_queues = [nc.sync, nc.scalar]

            def den_stage(isl):
                # denT [8, 512] = sum_c ksum_bd[:,c,:]^T @ qfT[:,c,slice]
                dz = dz_ps.tile([8, 512], F32, tag="dz")
                for c in range(4):
                    nc.tensor.matmul(
                        dz[:],
                        lhsT=ksum_bd[:, c, :],
                        rhs=qfT[:, c, isl * 512 : (isl + 1) * 512],
                        start=(c == 0),
                        stop=(c == 3),
                    )
                # Z = 1/(den+eps) as exp(-ln(den+eps)) on ACT; eps folds into
                # the Ln bias (Reciprocal is banned on ACT; DVE recip too slow)
                lntmp = lnt_pool.tile([8, 512], F32, tag="lntmp")
                nc.scalar.activation(lntmp[:], dz[:], AF.Ln, bias=eps_b[:])
                nc.scalar.activation(
                    ztsb[:, isl * 512 : (isl + 1) * 512], lntmp[:], AF.Exp, scale=-1.0
                )

            def stage1(iq):
                s0 = iq * 128
                # O^T per head pair (block-diagonal KV)
                otps = ot_ps.tile([128, 4, 128], F32, tag="ot")
                for c in range(4):
                    nc.tensor.matmul(
                        otps[:, c, :],
                        lhsT=kvsb[:, c, :],
                        rhs=qfT[:, c, s0 : s0 + 128],
                        start=True,
                        stop=True,
                    )
                # Zrep: replicate Z rows across head d-partitions (E8 selector)
                zrep = zr_ps.tile([128, 4, 128], F32, tag="zrep")
                for c in range(4):
                    nc.tensor.matmul(
                        zrep[:, c, :],
                        lhsT=e8[:, c, :],
                        rhs=ztsb[:, s0 : s0 + 128],
                        start=True,
                        stop=True,
                    )
                # DVE may read only one PSUM operand per op: stage zrep via ACT
                zrep_sb = zsb_pool.tile([128, 4, 128], BF16, tag="zrep_sb")
                nc.scalar.activation(zrep_sb[:], zrep[:], AF.Copy)
                otsb = osb_pool.tile([128, 4, 128], BF16, tag="otsb")
                nc.vector.tensor_mul(out=otsb[:], in0=otps[:], in1=zrep_sb[:])
                otsb_live[iq] = otsb

            def stage2(iq):
                otsb = otsb_live.pop(iq)
                # out = OT^T Wo; +bo' fused into the psum->sbuf copy
                owps = ow_ps.tile([128, 512], F32, tag="ow")
                for c in range(4):
                    nc.tensor.matmul(
                        owps[:],
                        lhsT=otsb[:, c, :],
                        rhs=w_sb["Wo"][:, c, :],
                        start=(c == 0),
                        stop=(c == 3),
                    )
                outt = out_pool.tile([128, 512], F32, tag="outt")
                nc.vector.tensor_add(out=outt[:], in0=owps[:], in1=bo2_rep[:])
                outq = out_queues[iq % 2]
                outq.dma_start(out=out_d[iq * 128 : (iq + 1) * 128, :], in_=outt[:])

            # quarter-slice main loop (128-wide: 1 PSUM bank per tile so
            # everything double-buffers within the 8 banks); OW skewed two
            # iterations behind OT/zrep so the zrep->ACT->DVE chain overlaps
            NQS = s_total // 128
            den_stage(0)
            den_stage(1)
            for iq in range(NQS + 2):
                if iq < NQS:
                    stage1(iq)
                if iq % 4 == 3 and (iq // 4 + 2) < NSL:
                    den_stage(iq // 4 + 2)
                if iq >= 2:
                    stage2(iq - 2)

    if reps == 1:
        _passes()
    else:
        with tc.For_i(0, reps, 1):
            _passes()


def _legalize_waits(nc: "bass.Bass", max_waits: int = 1) -> int:
    """This toolchain's walrus allows at most ONE sync wait per instruction.

    Tile's scheduler attaches several; hoist the extras into standalone
    event-semaphore (pure wait) instructions on the same engine, placed
    immediately before the original — identical blocking semantics since
    waits execute in stream order on the issuing sequencer.
    """
    n_split = 0
    for func in nc.m.functions:
        for block in func.blocks:
            new_insts = []
            for inst in block.instructions:
                si = getattr(inst, "sync_info", None)
                waits = list(si.on_wait) if (si and si.on_wait) else []
                if len(waits) > max_waits:
                    extra, keep = waits[:-max_waits], waits[-max_waits:]
                    for j, w in enumerate(extra):
                        ev = mybir.InstEventSemaphore(
                            name=f"{inst.name}_lw{j}",
                            engine=inst.engine,
                            ins=[],
                            outs=[],
                            sync_info=mybir.SyncInfo(on_wait=[w], on_update=[]),
                        )
                        new_insts.append(ev)
                        n_split += 1
                    si.on_wait = keep
                new_insts.append(inst)
            block.instructions[:] = new_insts
    return n_split


def build_program(s_total: int = S, reps: int = 1) -> "bass.Bass":
    nc = bass.Bass("TRN2", target_bir_lowering=False, debug=False, num_devices=B)
    io = {
        "xT": nc.dram_tensor("xT", [D, s_total], BF16, kind="ExternalInput").ap(),
        "Wq": nc.dram_tensor("Wq", [D, INNER], BF16, kind="ExternalInput").ap(),
        "bq": nc.dram_tensor("bq", [INNER], F32, kind="ExternalInput").ap(),
        "Wk": nc.dram_tensor("Wk", [D, INNER], BF16, kind="ExternalInput").ap(),
        "bk": nc.dram_tensor("bk", [INNER], BF16, kind="ExternalInput").ap(),
        "Wv": nc.dram_tensor("Wv", [D, INNER], BF16, kind="ExternalInput").ap(),
        "Wo": nc.dram_tensor("Wo", [INNER, D], BF16, kind="ExternalInput").ap(),
        "bo2": nc.dram_tensor("bo2", [D], F32, kind="ExternalInput").ap(),
        "out": nc.dram_tensor("out", [s_total, D], F32, kind="ExternalOutput").ap(),
    }
    with tile.TileContext(nc) as tc:
        with ExitStack() as ctx:
            _linattn_body(ctx, tc, io, s_total, reps=reps)
    return nc


_PROGRAM_CACHE: dict = {}


def _get_program(s_total: int = S) -> "bass.Bass":
    if s_total not in _PROGRAM_CACHE:
        nc = build_program(s_total)
        _legalize_waits(nc)
        _PROGRAM_CACHE[s_total] = nc
    return _PROGRAM_CACHE[s_total]


def _in_maps(inputs: dict) -> list:
    # pre-transpose x on the host: the device wants [d, s] for every matmul
    x_bf = np.asarray(inputs["x"]).astype(NP_BF16).transpose(0, 2, 1)
    w_bf = {
        nm: np.ascontiguousarray(np.asarray(inputs[nm]).astype(NP_BF16))
        for nm in ("Wq", "Wk", "Wv", "Wo")
    }
    bq = np.ascontiguousarray(inputs["bq"], dtype=np.float32)
    bk = np.ascontiguousarray(np.asarray(inputs["bk"]).astype(NP_BF16))
    bo2 = (
        np.asarray(inputs["bv"], dtype=np.float64) @ np.asarray(inputs["Wo"], dtype=np.float64)
        + np.asarray(inputs["bo"], dtype=np.float64)
    ).astype(np.float32)
    maps = []
    for b in range(B):
        m = {"xT": np.ascontiguousarray(x_bf[b]), "bq": bq, "bk": bk, "bo2": bo2}
        m.update(w_bf)
        maps.append(m)
    return maps


def run_hw(inputs: dict, trace: bool = False, **kwargs):
    """Run on the 8 NeuronCores. Returns (out [B,S,D], BassKernelResults)."""
    nc = _get_program(S)
    res = run_bass_kernel_spmd(nc, _in_maps(inputs), list(range(B)), trace=trace, **kwargs)
    out = np.stack([res.results[b]["out"] for b in range(B)], axis=0)
    return out, res


def kernel(**inputs) -> np.ndarray:
    out, _ = run_hw(inputs, trace=False)
    return out


def bench_hw(inputs: dict, iters: int = 20, nc_override=None):
    """Time repeated NEFF executions with device-resident inputs.

    Returns (per_iter_ns, out[B,S,D] from the first run). Uses the same
    shard_map lowering as run_bass_via_pjrt, without donation so input
    buffers can be reused across timed calls.
    """
    import time as _time

    import jax
    from jax.sharding import Mesh, NamedSharding, PartitionSpec
    from jax.experimental.shard_map import shard_map

    from concourse import bass2jax
    from concourse.bass2jax import _bass_exec_p, install_neuronx_cc_hook

    install_neuronx_cc_hook()
    nc = nc_override if nc_override is not None else _get_program(S)
    in_maps = _in_maps(inputs)

    partition_name = nc.partition_id_tensor.name if nc.partition_id_tensor else None
    in_names, out_names, out_avals = [], [], []
    for alloc in nc.m.functions[0].allocations:
        if not isinstance(alloc, mybir.MemoryLocationSet):
            continue
        name = alloc.memorylocations[0].name
        if alloc.kind == "ExternalInput":
            if name != partition_name:
                in_names.append(name)
        elif alloc.kind == "ExternalOutput":
            out_names.append(name)
            out_avals.append(
                jax.core.ShapedArray(tuple(alloc.tensor_shape), mybir.dt.np(alloc.dtype))
            )
    n_params = len(in_names)
    all_in_names = in_names + out_names
    if partition_name is not None:
        all_in_names = all_in_names + [partition_name]

    def _body(*args):
        operands = list(args)
        if partition_name is not None:
            operands.append(bass2jax.partition_id_tensor())
        outs = _bass_exec_p.bind(
            *operands,
            out_avals=tuple(out_avals),
            in_names=tuple(all_in_names),
            out_names=tuple(out_names),
            lowering_input_output_aliases=(),
            sim_require_finite=True,
            sim_require_nnan=True,
            nc=nc,
        )
        return tuple(outs)

    devices = jax.devices()[:B]
    mesh = Mesh(np.asarray(devices), ("core",))
    n_outs = len(out_names)
    in_specs = (PartitionSpec("core"),) * (n_params + n_outs)
    out_specs = (PartitionSpec("core"),) * n_outs
    fn = jax.jit(
        shard_map(_body, mesh=mesh, in_specs=in_specs, out_specs=out_specs, check_rep=False)
    )

    sh = NamedSharding(mesh, PartitionSpec("core"))
    concat_in = [
        jax.device_put(
            np.concatenate([np.asarray(in_maps[c][nm])[None] for c in range(B)], axis=0).reshape(
                B * np.asarray(in_maps[0][nm]).shape[0], *np.asarray(in_maps[0][nm]).shape[1:]
            ),
            sh,
        )
        for nm in in_names
    ]
    concat_zeros = [
        jax.device_put(np.zeros((B * a.shape[0], *a.shape[1:]), a.dtype), sh) for a in out_avals
    ]

    out = fn(*concat_in, *concat_zeros)
    jax.block_until_ready(out)
    first = np.asarray(out[0]).reshape(B, *out_avals[0].shape)

    def timed(f, n):
        t0 = _time.perf_counter()
        for _ in range(n):
            r = f(*concat_in, *concat_zeros)
        jax.block_until_ready(r)
        return (_time.perf_counter() - t0) / n

    timed(fn, 3)
    t = min(timed(fn, max(5, iters // 2)) for _ in range(4))
    return int(t * 1e9), first


def build_copy_program(s_total: int = S) -> "bass.Bass":
    """Same I/O signature as the real program, near-zero work: out = x upcast."""
    nc = bass.Bass("TRN2", target_bir_lowering=False, debug=False, num_devices=B)
    io = {}
    io["xT"] = nc.dram_tensor("xT", [D, s_total], BF16, kind="ExternalInput").ap()
    for nm, shp, dt in (
        ("Wq", [D, INNER], BF16), ("bq", [INNER], F32), ("Wk", [D, INNER], BF16),
        ("bk", [INNER], BF16), ("Wv", [D, INNER], BF16), ("Wo", [INNER, D], BF16),
        ("bo2", [D], F32),
    ):
        io[nm] = nc.dram_tensor(nm, shp, dt, kind="ExternalInput").ap()
    out_d = nc.dram_tensor("out", [s_total, D], F32, kind="ExternalOutput").ap()
    from contextlib import ExitStack as _ES
    with tile.TileContext(nc) as tc:
        with _ES() as ctx:
            pool = ctx.enter_context(tc.tile_pool(name="cp", bufs=4))
            z = pool.tile([128, D], F32, tag="z")
            nc.gpsimd.memset(z[:], 0.0)
            for i in range(s_total // 128):
                nc.sync.dma_start(out=out_d[i * 128 : (i + 1) * 128, :], in_=z[:])
    _legalize_waits(nc)
    return nc
